# revision 1
# baseline (speedup 1.0000x reference)
"""GCN encoder (2-layer GCNConv + tanh) on 8 Trainium2 NeuronCores.

Math refactor: with norm[e] = dinv[src]*dinv[dst], each GCNConv layer
    conv(h, W, b) = dinv * segsum((dinv * (h @ W))[src]) + b
                  = dinv * (segsum((dinv * h)[src]) @ W) + b
so the per-edge work is a pure gather + segment-sum of rows of a pre-scaled
bf16 table; dinv scalings, matmuls and biases are dense shard-local ops.

Sharding: nodes are permuted (serpentine degree balancing) and dealt into
8 cores x 49 blocks x 128 nodes. Edges are partitioned by destination
block; each block's incoming edges are padded to a fixed compile-time
number of 128-slot chunks. Each chunk is aggregated on the TensorEngine:
PSUM[feat, dst] += msgs[slot, feat].T @ onehot[slot, dst], where the
one-hot is built on VectorE via is_equal against an iota row.

The gather uses the SWDGE dma_gather instruction (int16 indices, max
32768 rows per call), so edges are split into stream A (table rows
< 32768) and stream B (rows >= 32768) with separate gather calls.

Halo exchange: AllGather of the bf16 table shards between layers.
"""
import sys
import os
import numpy as np

sys.path.insert(0, "/opt/trn_rl_repo")

import ml_dtypes  # noqa: E402
from contextlib import ExitStack  # noqa: E402

from concourse import bass, bacc, tile, mybir  # noqa: E402
from concourse.bass_utils import run_bass_kernel_spmd  # noqa: E402

N_NODES = 50000
N_EDGES = 800000
D = 128
NCORES = 8
BLK = 128
NBLK_PC = 49                 # blocks per core
SHARD = BLK * NBLK_PC        # 6272 rows per core
NPAD = NCORES * SHARD        # 50176
NBLK = NCORES * NBLK_PC      # 392
SPLIT = 32768                # stream A: table rows [0, SPLIT); B: [SPLIT, NPAD)
GROUP = 7                    # blocks per gather call (49 = 7*7)

F32 = mybir.dt.float32
BF16 = mybir.dt.bfloat16
I16 = mybir.dt.int16


def _preprocess(edge_index):
    """Host-side index preprocessing: permutation, edge partitioning,
    padded slot layout, gather-index / dst-offset arrays."""
    src = np.concatenate([edge_index[0], np.arange(N_NODES, dtype=np.int64)])
    dst = np.concatenate([edge_index[1], np.arange(N_NODES, dtype=np.int64)])
    deg = np.bincount(dst, minlength=N_NODES)
    dinv_node = (1.0 / np.sqrt(deg.astype(np.float64))).astype(np.float32)

    # serpentine deal of nodes (sorted by degree desc) into NBLK blocks
    order = np.argsort(-deg, kind="stable")
    i = np.arange(N_NODES)
    rnd = i // NBLK
    j = i % NBLK
    blk = np.where(rnd % 2 == 0, j, NBLK - 1 - j)
    pos = blk * BLK + rnd                      # position within [0, NPAD)
    pos_of_node = np.empty(N_NODES, np.int64)
    pos_of_node[order] = pos
    node_of_pos = np.full(NPAD, -1, np.int64)
    node_of_pos[pos] = order

    dinv_pos = np.zeros(NPAD, np.float32)
    dinv_pos[pos_of_node] = dinv_node

    psrc = pos_of_node[src]
    pdst = pos_of_node[dst]
    blk_g = pdst // BLK                        # 0..391
    dstoff = (pdst % BLK).astype(np.float32)
    stream = (psrc >= SPLIT).astype(np.int64)  # 0=A, 1=B

    # sort edges by (block, stream, src) for grouping + gather locality
    key = (blk_g * 2 + stream) * (NPAD + 1) + psrc
    eorder = np.argsort(key, kind="stable")
    blk_s = blk_g[eorder]
    stream_s = stream[eorder]
    psrc_s = psrc[eorder]
    dstoff_s = dstoff[eorder]

    gid = blk_s * 2 + stream_s
    cnt = np.bincount(gid, minlength=NBLK * 2).reshape(NBLK, 2)
    capA = int(np.ceil(cnt[:, 0].max() / BLK))
    capB = max(1, int(np.ceil(cnt[:, 1].max() / BLK)))

    # rank of each edge within its (block, stream) group
    group_start = np.zeros(NBLK * 2, np.int64)
    group_start[1:] = np.cumsum(cnt.reshape(-1))[:-1]
    rank = np.arange(len(eorder)) - group_start[gid]

    caps = np.array([capA, capB]) * BLK
    core_s = blk_s // NBLK_PC
    b_s = blk_s % NBLK_PC
    slot = b_s * caps[stream_s] + rank        # slot within core-stream array

    lenA = NBLK_PC * capA * BLK
    lenB = NBLK_PC * capB * BLK
    gidxA = np.zeros((NCORES, lenA), np.int16)
    gidxB = np.zeros((NCORES, lenB), np.int16)
    dofA = np.full((NCORES, lenA), -1.0, np.float32)
    dofB = np.full((NCORES, lenB), -1.0, np.float32)

    mA = stream_s == 0
    gidxA[core_s[mA], slot[mA]] = psrc_s[mA].astype(np.int16)
    dofA[core_s[mA], slot[mA]] = dstoff_s[mA]
    mB = ~mA
    gidxB[core_s[mB], slot[mB]] = (psrc_s[mB] - SPLIT).astype(np.int16)
    dofB[core_s[mB], slot[mB]] = dstoff_s[mB]

    def wrap_idx(a):
        # [len] -> [128, len/16]: token i at [i%16, i//16], replicated x8
        w = a.reshape(-1, 16).T
        return np.ascontiguousarray(np.tile(w, (8, 1)))

    def wrap_dof(a):
        # [len] -> [128, nchunks]: slot s at [s%128, s//128]
        return np.ascontiguousarray(a.reshape(-1, BLK).T)

    meta = dict(capA=capA, capB=capB, pos_of_node=pos_of_node,
                node_of_pos=node_of_pos, dinv_pos=dinv_pos)
    percore = []
    for c in range(NCORES):
        percore.append(dict(
            gidxA=wrap_idx(gidxA[c]), gidxB=wrap_idx(gidxB[c]),
            dofA=wrap_dof(dofA[c]), dofB=wrap_dof(dofB[c]),
        ))
    return meta, percore


def _build(capA, capB):
    """Build + compile the 8-core Bass program for given chunk caps."""
    nc = bacc.Bacc("TRN2", target_bir_lowering=False, debug=False,
                   num_devices=NCORES, num_swdge_queues=1)

    lenA = NBLK_PC * capA * BLK
    lenB = NBLK_PC * capB * BLK

    x_sh = nc.dram_tensor("x_sh", [SHARD, D], F32, kind="ExternalInput")
    idxA_in = nc.dram_tensor("idxA", [128, lenA // 16], I16, kind="ExternalInput")
    idxB_in = nc.dram_tensor("idxB", [128, lenB // 16], I16, kind="ExternalInput")
    dofA_in = nc.dram_tensor("dofA", [128, NBLK_PC * capA], F32, kind="ExternalInput")
    dofB_in = nc.dram_tensor("dofB", [128, NBLK_PC * capB], F32, kind="ExternalInput")
    dinvcol_in = nc.dram_tensor("dinvcol", [128, NBLK_PC], F32, kind="ExternalInput")
    dinvrep_in = nc.dram_tensor("dinvrep", [128, SHARD], F32, kind="ExternalInput")
    iota_in = nc.dram_tensor("iota", [128, 128], BF16, kind="ExternalInput")
    W1_in = nc.dram_tensor("W1", [D, D], BF16, kind="ExternalInput")
    W2_in = nc.dram_tensor("W2", [D, D], BF16, kind="ExternalInput")
    b1_in = nc.dram_tensor("b1", [128, 1], F32, kind="ExternalInput")
    b2_in = nc.dram_tensor("b2", [128, 1], F32, kind="ExternalInput")
    out_ext = nc.dram_tensor("outT", [128, SHARD], F32, kind="ExternalOutput")

    rg = [list(range(NCORES))]

    with tile.TileContext(nc) as tc, ExitStack() as ctx:
        const = ctx.enter_context(tc.tile_pool(name="const", bufs=1))
        work = ctx.enter_context(tc.tile_pool(name="work", bufs=4))
        msgs_p = ctx.enter_context(tc.tile_pool(name="msgs", bufs=2))
        psum = ctx.enter_context(tc.tile_pool(name="psum", bufs=2, space="PSUM"))
        dram = ctx.enter_context(tc.tile_pool(name="dram", bufs=1, space="DRAM"))

        # ---- constants into SBUF (used across both layers) ----
        iota_t = const.tile([128, 128], BF16, tag="iota")
        nc.sync.dma_start(iota_t[:], iota_in[:])
        W1_t = const.tile([D, D], BF16, tag="W1")
        nc.sync.dma_start(W1_t[:], W1_in[:])
        W2_t = const.tile([D, D], BF16, tag="W2")
        nc.sync.dma_start(W2_t[:], W2_in[:])
        b1_t = const.tile([128, 1], F32, tag="b1")
        nc.sync.dma_start(b1_t[:], b1_in[:])
        b2_t = const.tile([128, 1], F32, tag="b2")
        nc.sync.dma_start(b2_t[:], b2_in[:])
        dofA_t = const.tile([128, NBLK_PC * capA], F32, tag="dofA")
        nc.sync.dma_start(dofA_t[:], dofA_in[:])
        dofB_t = const.tile([128, NBLK_PC * capB], F32, tag="dofB")
        nc.sync.dma_start(dofB_t[:], dofB_in[:])
        idxA_t = const.tile([128, lenA // 16], I16, tag="idxA")
        nc.sync.dma_start(idxA_t[:], idxA_in[:])
        idxB_t = const.tile([128, lenB // 16], I16, tag="idxB")
        nc.sync.dma_start(idxB_t[:], idxB_in[:])
        dinvcol_t = const.tile([128, NBLK_PC], F32, tag="dinvcol")
        nc.sync.dma_start(dinvcol_t[:], dinvcol_in[:])
        dinvrep_t = const.tile([128, SHARD], F32, tag="dinvrep")
        nc.sync.dma_start(dinvrep_t[:], dinvrep_in[:])

        T1_shard = dram.tile([SHARD, D], BF16)
        T1_full = dram.tile([NPAD, D], BF16)
        T2_shard = dram.tile([SHARD, D], BF16)
        T2_full = dram.tile([NPAD, D], BF16)
        T1_int = nc.dram_tensor("T1_int", [NPAD, D], BF16)
        T2_int = nc.dram_tensor("T2_int", [NPAD, D], BF16)

        # ---- phase 1: T1 shard = dinv * x (bf16), then AllGather ----
        for b in range(NBLK_PC):
            xt = work.tile([128, D], F32, tag="xt")
            nc.sync.dma_start(xt[:], x_sh[b * BLK:(b + 1) * BLK, :])
            tt = work.tile([128, D], BF16, tag="tt")
            nc.vector.tensor_scalar(tt[:], xt[:], dinvcol_t[:, b:b + 1], None,
                                    mybir.AluOpType.mult)
            nc.sync.dma_start(T1_shard[b * BLK:(b + 1) * BLK, :], tt[:])
        nc.gpsimd.collective_compute(
            "AllGather", mybir.AluOpType.bypass, replica_groups=rg,
            ins=[T1_shard.opt()], outs=[T1_full.opt()])
        nc.sync.dma_start(T1_int[:, :], T1_full[:])

        def gather_group(tbl, g, cap, idx_t, tagc):
            """Gather GROUP consecutive blocks' slots in <=1024-idx sub-calls
            (all against one table base, back to back)."""
            n = GROUP * cap * BLK
            m = msgs_p.tile([128, GROUP * cap * D], BF16, tag=f"msgs{tagc}")
            m3 = m[:].rearrange("p (b e) -> p b e", e=D)
            base = g * (n // 16)
            done = 0
            while done < n:
                sub = min(1024, n - done)
                nc.gpsimd.dma_gather(
                    out_ap=m[:, (done // 128) * D:].rearrange(
                        "p (b e) -> p b e", e=D)[:, : sub // 128, :],
                    in_ap=tbl,
                    idxs_ap=idx_t[:, base + done // 16: base + (done + sub) // 16],
                    num_idxs=sub, num_idxs_reg=sub, elem_size=D)
                done += sub
            return m3

        def aggregate_block(m3A, m3B, bb, b):
            """19 one-hot matmuls accumulating PSUM[feat, dst] for block b."""
            agg = psum.tile([128, 128], F32, tag="agg")
            for k in range(capA + capB):
                isA = k < capA
                kk = k if isA else k - capA
                m3, dof_t, cap = (m3A, dofA_t, capA) if isA else (m3B, dofB_t, capB)
                S = work.tile([128, 128], BF16, tag="S")
                nc.vector.tensor_scalar(S[:], iota_t[:],
                                        dof_t[:, b * cap + kk:b * cap + kk + 1],
                                        None, mybir.AluOpType.is_equal)
                nc.tensor.matmul(agg[:], lhsT=m3[:, bb * cap + kk, :], rhs=S[:],
                                 start=(k == 0), stop=(k == capA + capB - 1))
            return agg

        # ---- phase 2: layer 1 -> T2 shard, then AllGather ----
        tblA = T1_int[0:SPLIT, :]
        tblB = T1_int[SPLIT:NPAD, :]
        for g in range(NBLK_PC // GROUP):
            m3A = gather_group(tblA, g, capA, idxA_t, "A")
            m3B = gather_group(tblB, g, capB, idxB_t, "B")
            for bb in range(GROUP):
                b = g * GROUP + bb
                agg = aggregate_block(m3A, m3B, bb, b)
                dvs = dinvrep_t[:, b * BLK:(b + 1) * BLK]
                z1 = work.tile([128, 128], BF16, tag="z1")
                nc.vector.tensor_tensor(out=z1[:], in0=agg[:], in1=dvs,
                                        op=mybir.AluOpType.mult)
                h1T = psum.tile([128, 128], F32, tag="h1T")
                nc.tensor.matmul(h1T[:], lhsT=W1_t[:], rhs=z1[:], start=True, stop=True)
                u1 = work.tile([128, 128], BF16, tag="u1")
                nc.scalar.activation(u1[:], h1T[:],
                                     mybir.ActivationFunctionType.Identity,
                                     bias=b1_t[:, 0:1])
                h2pT = psum.tile([128, 128], F32, tag="h2pT")
                nc.tensor.matmul(h2pT[:], lhsT=W2_t[:], rhs=u1[:], start=True, stop=True)
                t2 = work.tile([128, 128], BF16, tag="t2")
                nc.vector.tensor_tensor(out=t2[:], in0=h2pT[:], in1=dvs,
                                        op=mybir.AluOpType.mult)
                t2T = work.tile([128, 128], BF16, tag="t2T")
                nc.sync.dma_start(t2T[:], t2[:], transpose=True)
                nc.sync.dma_start(T2_shard[b * BLK:(b + 1) * BLK, :], t2T[:])
        nc.gpsimd.collective_compute(
            "AllGather", mybir.AluOpType.bypass, replica_groups=rg,
            ins=[T2_shard.opt()], outs=[T2_full.opt()])
        nc.sync.dma_start(T2_int[:, :], T2_full[:])

        # ---- phase 3: layer 2 -> tanh -> output (feat-major) ----
        tblA2 = T2_int[0:SPLIT, :]
        tblB2 = T2_int[SPLIT:NPAD, :]
        for g in range(NBLK_PC // GROUP):
            m3A = gather_group(tblA2, g, capA, idxA_t, "A")
            m3B = gather_group(tblB2, g, capB, idxB_t, "B")
            for bb in range(GROUP):
                b = g * GROUP + bb
                agg = aggregate_block(m3A, m3B, bb, b)
                dvs = dinvrep_t[:, b * BLK:(b + 1) * BLK]
                v = work.tile([128, 128], F32, tag="v")
                nc.vector.tensor_tensor(out=v[:], in0=agg[:], in1=dvs,
                                        op=mybir.AluOpType.mult)
                ob = work.tile([128, 128], F32, tag="ob")
                nc.scalar.activation(ob[:], v[:],
                                     mybir.ActivationFunctionType.Tanh,
                                     bias=b2_t[:, 0:1])
                nc.sync.dma_start(out_ext[:, b * BLK:(b + 1) * BLK], ob[:])

    nc.compile()
    return nc


_CACHE = {}


def _get_nc(capA, capB):
    key = (capA, capB)
    if key not in _CACHE:
        _CACHE[key] = _build(capA, capB)
    return _CACHE[key]


def kernel(x, edge_index, W1, b1, W2, b2, _want_profile=False):
    x = np.asarray(x, np.float32)
    edge_index = np.asarray(edge_index)
    meta, percore = _preprocess(edge_index)
    capA, capB = meta["capA"], meta["capB"]
    nc = _get_nc(capA, capB)

    pos_of_node = meta["pos_of_node"]
    node_of_pos = meta["node_of_pos"]
    dinv_pos = meta["dinv_pos"]

    xp = np.zeros((NPAD, D), np.float32)
    xp[pos_of_node] = x
    iota = np.ascontiguousarray(
        np.broadcast_to(np.arange(128, dtype=np.float32), (128, 128))
    ).astype(ml_dtypes.bfloat16)
    W1b = np.asarray(W1, np.float32).astype(ml_dtypes.bfloat16)
    W2b = np.asarray(W2, np.float32).astype(ml_dtypes.bfloat16)
    b1c = np.ascontiguousarray(np.asarray(b1, np.float32).reshape(128, 1))
    b2c = np.ascontiguousarray(np.asarray(b2, np.float32).reshape(128, 1))

    in_maps = []
    for c in range(NCORES):
        lo = c * SHARD
        dinv_sh = dinv_pos[lo:lo + SHARD]
        in_maps.append({
            "x_sh": np.ascontiguousarray(xp[lo:lo + SHARD]),
            "idxA": percore[c]["gidxA"],
            "idxB": percore[c]["gidxB"],
            "dofA": percore[c]["dofA"],
            "dofB": percore[c]["dofB"],
            "dinvcol": np.ascontiguousarray(dinv_sh.reshape(NBLK_PC, BLK).T),
            "dinvrep": np.ascontiguousarray(
                np.broadcast_to(dinv_sh, (128, SHARD))),
            "iota": iota,
            "W1": W1b, "W2": W2b, "b1": b1c, "b2": b2c,
        })

    res = run_bass_kernel_spmd(nc, in_maps, list(range(NCORES)),
                               trace=_want_profile)
    full = np.concatenate([res.results[c]["outT"].T for c in range(NCORES)],
                          axis=0)
    out = full[pos_of_node]
    if _want_profile:
        return out, res
    return out



# revision 2
# speedup vs baseline: 3658.0953x; 3658.0953x over previous
"""GCN encoder (2-layer GCNConv + tanh) on 8 Trainium2 NeuronCores — v4.

Math refactor: with norm[e] = dinv[src]*dinv[dst], each GCNConv layer
    conv(h, W, b) = dinv * (segsum((dinv * h)[src]) @ W) + b
so the per-edge work is a pure gather + segment-sum of rows of a pre-scaled
bf16 table; dinv scalings, matmuls and biases are dense shard-local ops.

v4 vs v3b: the self-loops added by GCNConv are dropped from the gather
streams and instead contributed per block by one extra TensorE matmul
against a dinv-scaled diagonal, reading the block's rows from a retained
SBUF copy of the local table shard (they are core-local by construction).
This cuts gathered slots ~10% (caps 10,10 -> 9,9).

Retained v3 structure: per-core table shard split into two block-aligned
half-segments A (25 blocks) / B (24 blocks); two AllGathers per layer
directly into Shared DRAM tables so stream-A edge gathers overlap the
segment-B collective; segment-relative int16 gather indices; 4 SWDGE
queues round-robin across gather sub-calls.
"""
import sys
import os
import numpy as np

sys.path.insert(0, "/opt/trn_rl_repo")

import ml_dtypes  # noqa: E402
from contextlib import ExitStack  # noqa: E402

from concourse import bass, bacc, tile, mybir  # noqa: E402
from concourse.bass_utils import run_bass_kernel_spmd  # noqa: E402

N_NODES = 50000
N_EDGES = 800000
D = 128
NCORES = 8
BLK = 128
NBLK_PC = 49                 # blocks per core
SHARD = BLK * NBLK_PC        # 6272 rows per core
NPAD = NCORES * SHARD        # 50176
NBLK = NCORES * NBLK_PC      # 392
ABLK = 25                    # segment-A blocks per core
BBLK = NBLK_PC - ABLK        # 24
AROWS = ABLK * BLK           # 3200
BROWS = BBLK * BLK           # 3072
GROUP = 7                    # blocks per gather call (49 = 7*7)
SUB = 1024                   # gather sub-call size (= SWDGE ring)

F32 = mybir.dt.float32
BF16 = mybir.dt.bfloat16
I16 = mybir.dt.int16


def _preprocess(edge_index):
    """Host-side index preprocessing: permutation, edge partitioning,
    padded slot layout, gather-index / dst-offset arrays. The GCN self
    loops participate in deg/dinv but are handled on-device as local
    diagonal contributions, not gathered edges."""
    loops = np.arange(N_NODES, dtype=np.int64)
    dst_all = np.concatenate([edge_index[1], loops])
    deg = np.bincount(dst_all, minlength=N_NODES)
    dinv_node = (1.0 / np.sqrt(deg.astype(np.float64))).astype(np.float32)

    # serpentine deal of nodes (sorted by degree desc) into NBLK blocks
    order = np.argsort(-deg, kind="stable")
    i = np.arange(N_NODES)
    rnd = i // NBLK
    j = i % NBLK
    blk = np.where(rnd % 2 == 0, j, NBLK - 1 - j)
    pos = blk * BLK + rnd                      # position within [0, NPAD)
    pos_of_node = np.empty(N_NODES, np.int64)
    pos_of_node[order] = pos
    node_of_pos = np.full(NPAD, -1, np.int64)
    node_of_pos[pos] = order

    dinv_pos = np.zeros(NPAD, np.float32)
    dinv_pos[pos_of_node] = dinv_node

    src = edge_index[0]                        # real edges only
    dst = edge_index[1]
    psrc = pos_of_node[src]
    pdst = pos_of_node[dst]
    blk_g = pdst // BLK                        # 0..391
    dstoff = (pdst % BLK).astype(np.float32)

    # stream split by source half-segment (block-aligned within its core)
    csrc = psrc // SHARD
    rsrc = psrc % SHARD
    stream = (rsrc >= AROWS).astype(np.int64)  # 0=A, 1=B
    segrow = np.where(stream == 0, csrc * AROWS + rsrc,
                      csrc * BROWS + (rsrc - AROWS))

    # sort edges by (dst block, stream, src) for grouping + gather locality
    key = (blk_g * 2 + stream) * (NPAD + 1) + psrc
    eorder = np.argsort(key, kind="stable")
    blk_s = blk_g[eorder]
    stream_s = stream[eorder]
    segrow_s = segrow[eorder]
    dstoff_s = dstoff[eorder]

    gid = blk_s * 2 + stream_s
    cnt = np.bincount(gid, minlength=NBLK * 2).reshape(NBLK, 2)
    capA = max(1, int(np.ceil(cnt[:, 0].max() / BLK)))
    capB = max(1, int(np.ceil(cnt[:, 1].max() / BLK)))

    # rank of each edge within its (block, stream) group
    group_start = np.zeros(NBLK * 2, np.int64)
    group_start[1:] = np.cumsum(cnt.reshape(-1))[:-1]
    rank = np.arange(len(eorder)) - group_start[gid]

    caps = np.array([capA, capB]) * BLK
    core_s = blk_s // NBLK_PC
    b_s = blk_s % NBLK_PC
    slot = b_s * caps[stream_s] + rank        # slot within core-stream array

    lenA = NBLK_PC * capA * BLK
    lenB = NBLK_PC * capB * BLK
    gidxA = np.zeros((NCORES, lenA), np.int16)
    gidxB = np.zeros((NCORES, lenB), np.int16)
    dofA = np.full((NCORES, lenA), -1.0, np.float32)
    dofB = np.full((NCORES, lenB), -1.0, np.float32)

    mA = stream_s == 0
    gidxA[core_s[mA], slot[mA]] = segrow_s[mA].astype(np.int16)
    dofA[core_s[mA], slot[mA]] = dstoff_s[mA]
    mB = ~mA
    gidxB[core_s[mB], slot[mB]] = segrow_s[mB].astype(np.int16)
    dofB[core_s[mB], slot[mB]] = dstoff_s[mB]

    def wrap_idx(a):
        # [len] -> [128, len/16]: token i at [i%16, i//16], replicated x8
        w = a.reshape(-1, 16).T
        return np.ascontiguousarray(np.tile(w, (8, 1)))

    def wrap_dof(a):
        # [len] -> [128, nchunks]: slot s at [s%128, s//128]
        return np.ascontiguousarray(a.reshape(-1, BLK).T)

    meta = dict(capA=capA, capB=capB, pos_of_node=pos_of_node,
                node_of_pos=node_of_pos, dinv_pos=dinv_pos)
    percore = []
    for c in range(NCORES):
        percore.append(dict(
            gidxA=wrap_idx(gidxA[c]), gidxB=wrap_idx(gidxB[c]),
            dofA=wrap_dof(dofA[c]), dofB=wrap_dof(dofB[c]),
        ))
    return meta, percore


def _build(capA, capB):
    """Build + compile the 8-core Bass program for given chunk caps."""
    nc = bacc.Bacc("TRN2", target_bir_lowering=False, debug=False,
                   num_devices=NCORES, num_swdge_queues=4)

    lenA = NBLK_PC * capA * BLK
    lenB = NBLK_PC * capB * BLK

    x_sh = nc.dram_tensor("x_sh", [SHARD, D], F32, kind="ExternalInput")
    idxA_in = nc.dram_tensor("idxA", [128, lenA // 16], I16, kind="ExternalInput")
    idxB_in = nc.dram_tensor("idxB", [128, lenB // 16], I16, kind="ExternalInput")
    dofA_in = nc.dram_tensor("dofA", [128, NBLK_PC * capA], F32, kind="ExternalInput")
    dofB_in = nc.dram_tensor("dofB", [128, NBLK_PC * capB], F32, kind="ExternalInput")
    dinvcol_in = nc.dram_tensor("dinvcol", [128, NBLK_PC], F32, kind="ExternalInput")
    dinvrep_in = nc.dram_tensor("dinvrep", [128, SHARD], F32, kind="ExternalInput")
    iota_in = nc.dram_tensor("iota", [128, 128], BF16, kind="ExternalInput")
    diag_in = nc.dram_tensor("diag", [128, 128], BF16, kind="ExternalInput")
    W1_in = nc.dram_tensor("W1", [D, D], BF16, kind="ExternalInput")
    W2_in = nc.dram_tensor("W2", [D, D], BF16, kind="ExternalInput")
    b1_in = nc.dram_tensor("b1", [128, 1], F32, kind="ExternalInput")
    b2_in = nc.dram_tensor("b2", [128, 1], F32, kind="ExternalInput")
    out_ext = nc.dram_tensor("outT", [128, SHARD], F32, kind="ExternalOutput")

    rg = [list(range(NCORES))]

    with tile.TileContext(nc) as tc, ExitStack() as ctx:
        const = ctx.enter_context(tc.tile_pool(name="const", bufs=1))
        work = ctx.enter_context(tc.tile_pool(name="work", bufs=4))
        msgs_p = ctx.enter_context(tc.tile_pool(name="msgs", bufs=2))
        psum = ctx.enter_context(tc.tile_pool(name="psum", bufs=2, space="PSUM"))
        dram = ctx.enter_context(tc.tile_pool(name="dram", bufs=1, space="DRAM"))

        # ---- constants into SBUF (used across both layers) ----
        iota_t = const.tile([128, 128], BF16, tag="iota")
        nc.sync.dma_start(iota_t[:], iota_in[:])
        diag_t = const.tile([128, 128], BF16, tag="diag")
        nc.sync.dma_start(diag_t[:], diag_in[:])
        W1_t = const.tile([D, D], BF16, tag="W1")
        nc.sync.dma_start(W1_t[:], W1_in[:])
        W2_t = const.tile([D, D], BF16, tag="W2")
        nc.sync.dma_start(W2_t[:], W2_in[:])
        b1_t = const.tile([128, 1], F32, tag="b1")
        nc.sync.dma_start(b1_t[:], b1_in[:])
        b2_t = const.tile([128, 1], F32, tag="b2")
        nc.sync.dma_start(b2_t[:], b2_in[:])
        dofA_t = const.tile([128, NBLK_PC * capA], F32, tag="dofA")
        nc.sync.dma_start(dofA_t[:], dofA_in[:])
        dofB_t = const.tile([128, NBLK_PC * capB], F32, tag="dofB")
        nc.sync.dma_start(dofB_t[:], dofB_in[:])
        idxA_t = const.tile([128, lenA // 16], I16, tag="idxA")
        nc.sync.dma_start(idxA_t[:], idxA_in[:])
        idxB_t = const.tile([128, lenB // 16], I16, tag="idxB")
        nc.sync.dma_start(idxB_t[:], idxB_in[:])
        dinvcol_t = const.tile([128, NBLK_PC], F32, tag="dinvcol")
        nc.sync.dma_start(dinvcol_t[:], dinvcol_in[:])
        dinvrep_t = const.tile([128, SHARD], F32, tag="dinvrep")
        nc.sync.dma_start(dinvrep_t[:], dinvrep_in[:])

        # retained local tables (node-major rows of own shard, bf16)
        T1loc = const.tile([128, SHARD], BF16, tag="T1loc")
        T2loc = const.tile([128, SHARD], BF16, tag="T2loc")

        T1_shA = dram.tile([AROWS, D], BF16)
        T1_shB = dram.tile([BROWS, D], BF16)
        T2_shA = dram.tile([AROWS, D], BF16)
        T2_shB = dram.tile([BROWS, D], BF16)
        T1_A = nc.dram_tensor("T1_A", [NCORES * AROWS, D], BF16,
                              addr_space="Shared")
        T1_B = nc.dram_tensor("T1_B", [NCORES * BROWS, D], BF16,
                              addr_space="Shared")
        T2_A = nc.dram_tensor("T2_A", [NCORES * AROWS, D], BF16,
                              addr_space="Shared")
        T2_B = nc.dram_tensor("T2_B", [NCORES * BROWS, D], BF16,
                              addr_space="Shared")

        def scale_block(b, dst_tile, dst_off):
            xt = work.tile([128, D], F32, tag="xt")
            nc.sync.dma_start(xt[:], x_sh[b * BLK:(b + 1) * BLK, :])
            nc.vector.tensor_scalar(T1loc[:, b * BLK:(b + 1) * BLK],
                                    xt[:], dinvcol_t[:, b:b + 1], None,
                                    mybir.AluOpType.mult)
            nc.sync.dma_start(dst_tile[dst_off:dst_off + BLK, :],
                              T1loc[:, b * BLK:(b + 1) * BLK])

        # ---- phase 1: T1 shard halves = dinv * x (bf16) + AllGathers ----
        for b in range(ABLK):
            scale_block(b, T1_shA, b * BLK)
        nc.gpsimd.collective_compute(
            "AllGather", mybir.AluOpType.bypass, replica_groups=rg,
            ins=[T1_shA.opt()], outs=[T1_A[:, :]])
        for b in range(ABLK, NBLK_PC):
            scale_block(b, T1_shB, (b - ABLK) * BLK)
        nc.gpsimd.collective_compute(
            "AllGather", mybir.AluOpType.bypass, replica_groups=rg,
            ins=[T1_shB.opt()], outs=[T1_B[:, :]])

        qctr = [0]

        def gather_group(tbl, g, cap, idx_t, tagc):
            """Gather GROUP consecutive blocks' slots in <=SUB-idx sub-calls
            (all against one table base, back to back), round-robin across
            SWDGE queues so desc-gen overlaps the previous call's drain."""
            n = GROUP * cap * BLK
            m = msgs_p.tile([128, GROUP * cap * D], BF16, tag=f"msgs{tagc}")
            m3 = m[:].rearrange("p (b e) -> p b e", e=D)
            base = g * (n // 16)
            done = 0
            while done < n:
                sub = min(SUB, n - done)
                nc.gpsimd.dma_gather(
                    out_ap=m[:, (done // 128) * D:].rearrange(
                        "p (b e) -> p b e", e=D)[:, : sub // 128, :],
                    in_ap=tbl,
                    idxs_ap=idx_t[:, base + done // 16: base + (done + sub) // 16],
                    num_idxs=sub, num_idxs_reg=sub, elem_size=D,
                    queue_num=qctr[0] % 4)
                qctr[0] += 1
                done += sub
            return m3

        def aggregate_block(m3A, m3B, bb, b, loc):
            """Gathered-chunk one-hot matmuls + local self-loop diagonal,
            accumulating PSUM[feat, dst] for block b."""
            agg = psum.tile([128, 128], F32, tag="agg")
            # self-loop term: agg[f, d] += T_loc[d, f]; the dinv[dst] factor
            # comes from the post-scale, dinv[src]==dinv[dst] is in the table
            nc.tensor.matmul(agg[:], lhsT=loc[:, b * BLK:(b + 1) * BLK],
                             rhs=diag_t[:], start=True, stop=False)
            for k in range(capA + capB):
                isA = k < capA
                kk = k if isA else k - capA
                m3, dof_t, cap = (m3A, dofA_t, capA) if isA else (m3B, dofB_t, capB)
                S = work.tile([128, 128], BF16, tag="S")
                nc.vector.tensor_scalar(S[:], iota_t[:],
                                        dof_t[:, b * cap + kk:b * cap + kk + 1],
                                        None, mybir.AluOpType.is_equal)
                nc.tensor.matmul(agg[:], lhsT=m3[:, bb * cap + kk, :], rhs=S[:],
                                 start=False, stop=(k == capA + capB - 1))
            return agg

        # ---- phase 2: layer 1 -> T2 shard halves + AllGathers ----
        for g in range(NBLK_PC // GROUP):
            m3A = gather_group(T1_A[:, :], g, capA, idxA_t, "A")
            m3B = gather_group(T1_B[:, :], g, capB, idxB_t, "B")
            for bb in range(GROUP):
                b = g * GROUP + bb
                agg = aggregate_block(m3A, m3B, bb, b, T1loc)
                dvs = dinvrep_t[:, b * BLK:(b + 1) * BLK]
                z1 = work.tile([128, 128], BF16, tag="z1")
                nc.vector.tensor_tensor(out=z1[:], in0=agg[:], in1=dvs,
                                        op=mybir.AluOpType.mult)
                h1T = psum.tile([128, 128], F32, tag="h1T")
                nc.tensor.matmul(h1T[:], lhsT=W1_t[:], rhs=z1[:], start=True, stop=True)
                u1 = work.tile([128, 128], BF16, tag="u1")
                nc.scalar.activation(u1[:], h1T[:],
                                     mybir.ActivationFunctionType.Identity,
                                     bias=b1_t[:, 0:1])
                h2pT = psum.tile([128, 128], F32, tag="h2pT")
                nc.tensor.matmul(h2pT[:], lhsT=W2_t[:], rhs=u1[:], start=True, stop=True)
                t2 = work.tile([128, 128], BF16, tag="t2")
                nc.vector.tensor_tensor(out=t2[:], in0=h2pT[:], in1=dvs,
                                        op=mybir.AluOpType.mult)
                nc.sync.dma_start(T2loc[:, b * BLK:(b + 1) * BLK], t2[:],
                                  transpose=True)
                if b < ABLK:
                    nc.sync.dma_start(T2_shA[b * BLK:(b + 1) * BLK, :],
                                      T2loc[:, b * BLK:(b + 1) * BLK])
                else:
                    nc.sync.dma_start(
                        T2_shB[(b - ABLK) * BLK:(b - ABLK + 1) * BLK, :],
                        T2loc[:, b * BLK:(b + 1) * BLK])
                if b == ABLK - 1:
                    nc.gpsimd.collective_compute(
                        "AllGather", mybir.AluOpType.bypass, replica_groups=rg,
                        ins=[T2_shA.opt()], outs=[T2_A[:, :]])
        nc.gpsimd.collective_compute(
            "AllGather", mybir.AluOpType.bypass, replica_groups=rg,
            ins=[T2_shB.opt()], outs=[T2_B[:, :]])

        # ---- phase 3: layer 2 -> tanh -> output (feat-major) ----
        for g in range(NBLK_PC // GROUP):
            m3A = gather_group(T2_A[:, :], g, capA, idxA_t, "A")
            m3B = gather_group(T2_B[:, :], g, capB, idxB_t, "B")
            for bb in range(GROUP):
                b = g * GROUP + bb
                agg = aggregate_block(m3A, m3B, bb, b, T2loc)
                dvs = dinvrep_t[:, b * BLK:(b + 1) * BLK]
                v = work.tile([128, 128], F32, tag="v")
                nc.vector.tensor_tensor(out=v[:], in0=agg[:], in1=dvs,
                                        op=mybir.AluOpType.mult)
                ob = work.tile([128, 128], F32, tag="ob")
                nc.scalar.activation(ob[:], v[:],
                                     mybir.ActivationFunctionType.Tanh,
                                     bias=b2_t[:, 0:1])
                nc.sync.dma_start(out_ext[:, b * BLK:(b + 1) * BLK], ob[:])

    nc.compile()
    return nc


_CACHE = {}


def _get_nc(capA, capB):
    key = (capA, capB)
    if key not in _CACHE:
        _CACHE[key] = _build(capA, capB)
    return _CACHE[key]


def make_in_maps(x, W1, b1, W2, b2, meta, percore):
    pos_of_node = meta["pos_of_node"]
    dinv_pos = meta["dinv_pos"]

    xp = np.zeros((NPAD, D), np.float32)
    xp[pos_of_node] = np.asarray(x, np.float32)
    iota = np.ascontiguousarray(
        np.broadcast_to(np.arange(128, dtype=np.float32), (128, 128))
    ).astype(ml_dtypes.bfloat16)
    diag = np.eye(128, dtype=np.float32).astype(ml_dtypes.bfloat16)
    W1b = np.asarray(W1, np.float32).astype(ml_dtypes.bfloat16)
    W2b = np.asarray(W2, np.float32).astype(ml_dtypes.bfloat16)
    b1c = np.ascontiguousarray(np.asarray(b1, np.float32).reshape(128, 1))
    b2c = np.ascontiguousarray(np.asarray(b2, np.float32).reshape(128, 1))

    in_maps = []
    for c in range(NCORES):
        lo = c * SHARD
        dinv_sh = dinv_pos[lo:lo + SHARD]
        in_maps.append({
            "x_sh": np.ascontiguousarray(xp[lo:lo + SHARD]),
            "idxA": percore[c]["gidxA"],
            "idxB": percore[c]["gidxB"],
            "dofA": percore[c]["dofA"],
            "dofB": percore[c]["dofB"],
            "dinvcol": np.ascontiguousarray(dinv_sh.reshape(NBLK_PC, BLK).T),
            "dinvrep": np.ascontiguousarray(
                np.broadcast_to(dinv_sh, (128, SHARD))),
            "iota": iota,
            "diag": diag,
            "W1": W1b, "W2": W2b, "b1": b1c, "b2": b2c,
        })
    return in_maps


def kernel(x, edge_index, W1, b1, W2, b2, _want_profile=False):
    x = np.asarray(x, np.float32)
    edge_index = np.asarray(edge_index)
    meta, percore = _preprocess(edge_index)
    capA, capB = meta["capA"], meta["capB"]
    nc = _get_nc(capA, capB)
    in_maps = make_in_maps(x, W1, b1, W2, b2, meta, percore)

    res = run_bass_kernel_spmd(nc, in_maps, list(range(NCORES)),
                               trace=_want_profile)
    full = np.concatenate([res.results[c]["outT"].T for c in range(NCORES)],
                          axis=0)
    out = full[meta["pos_of_node"]]
    if _want_profile:
        return out, res
    return out
